# revision 2
# baseline (speedup 1.0000x reference)
"""Additive (Bahdanau) attention kernel for Trainium2, 8 NeuronCores — v2.

Math (per batch b):
  Wv = v @ W            [Tv, D]
  Uh = h @ U            [Th, D]
  q[s,t] = sum_d w[d] * tanh(Uh[s,d] + Wv[t,d] + b[d])
  beta = softmax_t(q)
  u = beta @ v          [Th, F]

v2 core idea: replace tanh with a K-term sine fit
  tanh(x) ~= sum_k C_k sin(OM_k x)
and expand sin(OM(A+B)) = sin(OM A)cos(OM B) + cos(OM A)sin(OM B).  The huge
[s,t,d] broadcast+tanh tensor (the v1 bottleneck: ~27us ScalarE + ~17us DVE
per core) collapses into per-side trig evals on [s,d]x K and [t,d]x K plus
TensorE contractions over (d,k).  q lands in PSUM as [t,s], so softmax+context
need no transposes.

HW Sin is only valid on ~[-pi,pi], so arguments (up to ~21 rad) are
range-reduced in period units with the bf16 magic-number rounding trick:
  tmp  = bf16(Xp + (192+phi))      # write-cast rounds to integer quantum 1
  -r   = (tmp - (192+phi)) - Xp    # scalar_tensor_tensor
  trig = Sin(-r, scale=-2pi)       # = sin(2pi(Xp+phi) - 2pi round(.))
phi = 0 gives sin, phi = 0.25 gives cos.  All on DVE at 4x/2x bf16 rates.

Sharding: pure data-parallel over B (16 batches -> 2 per core), weights
replicated, no collectives.  Host-side staging ships every operand
pre-layouted and pre-cast (bf16 except f32 biases).
"""

import ml_dtypes
import numpy as np

B, TV, TH, F, H, D = 16, 128, 64, 512, 512, 256
NCORES = 8
BL = B // NCORES  # 2 batches per core
DCN = 2  # d chunks of 128
FCN = 4
HCN = 4
K = 5

# tanh(x) ~= sum C_k sin(2*pi*OMP_k x); free-frequency fit tuned end-to-end
# under exact kernel numerics.  OMP (period units) is bf16-exact.
OMP = np.array([0.04833984375, 0.099609375, 0.1826171875, 0.302734375,
                0.451171875], np.float32)
C = np.array([1.116507887840271, 0.22106504440307617, 0.2375125288963318,
              0.08239565789699554, 0.025441491976380348], np.float32)
PI = float(np.pi)
MAGIC = 192.0

_CACHE = {}
BF16 = ml_dtypes.bfloat16


def _split_excess_waits(nc, mybir):
    """Walrus rejects instructions carrying >1 sync-wait; split extras onto
    same-engine NoOp carriers."""
    EXEMPT = ("InstUnconditionalBranch", "InstCall")
    k = 0
    for f in nc.m.functions:
        for blk in f.blocks:
            insts = list(blk.instructions)
            out, changed = [], False
            for inst in insts:
                si = inst.sync_info
                tn = type(inst).__name__
                if (si is not None and si.on_wait and len(si.on_wait) > 1
                        and tn not in EXEMPT):
                    waits = list(si.on_wait)
                    for wext in waits[:-1]:
                        noop = mybir.InstNoOp(name=f"wsplit-{k}")
                        k += 1
                        noop.engine = inst.engine
                        noop.sync_info = mybir.SyncInfo(
                            on_wait=[wext], on_update=[]
                        )
                        out.append(noop)
                    inst.sync_info = mybir.SyncInfo(
                        on_wait=waits[-1:], on_update=list(si.on_update or [])
                    )
                    changed = True
                out.append(inst)
            if changed:
                blk.instructions = out


def _build_nc():
    import concourse.bass as bass
    import concourse.tile as tile
    from concourse import mybir

    f32 = mybir.dt.float32
    bf16 = mybir.dt.bfloat16
    AF = mybir.ActivationFunctionType
    AL = mybir.AluOpType

    CW = 2 * (2 + K + DCN * K)  # consts width in bf16 units
    B0W = CW + HCN * BL * TH + HCN * 128   # consts | hT | Uc-ch0
    B1W = FCN * BL * TV + FCN * 128        # vT | Wc-ch0
    nc = bass.Bass()
    b0_e = nc.declare_dram_parameter("b0", [128, B0W], bf16, isOutput=False)
    uc1_e = nc.declare_dram_parameter("uc1", [128, HCN, 128], bf16, isOutput=False)
    b1_e = nc.declare_dram_parameter("b1", [128, B1W], bf16, isOutput=False)
    wc1_e = nc.declare_dram_parameter("wc1", [128, FCN, 128], bf16, isOutput=False)
    vN_e = nc.declare_dram_parameter("vN", [128, BL, F], bf16, isOutput=False)
    out_e = nc.declare_dram_parameter("out", [BL, TH, F], f32, isOutput=True)

    with tile.TileContext(nc) as tc:
        with (
            tc.tile_pool(name="consts", bufs=1) as consts,
            tc.tile_pool(name="work", bufs=1) as work,
            tc.tile_pool(name="ps_pu", bufs=1, space="PSUM") as ps_pu,
            tc.tile_pool(name="ps_pw", bufs=1, space="PSUM") as ps_pw,
            tc.tile_pool(name="ps_q", bufs=2, space="PSUM") as ps_q,
            tc.tile_pool(name="ps_u", bufs=2, space="PSUM") as ps_u,
            tc.tile_pool(name="ps_d", bufs=1, space="PSUM") as ps_d,
        ):
            # ---------------- loads ----------------
            zz = consts.tile([128, 1], f32)
            nc.gpsimd.memset(zz[:], 0.0)
            ones_t = consts.tile([128, 1], bf16)
            nc.gpsimd.memset(ones_t[:], 1.0)
            pihalf = consts.tile([128, 1], f32)
            nc.gpsimd.memset(pihalf[:], PI / 2)
            # touch ACT so the Sin table set loads off the critical path
            scrap = consts.tile([128, 1], f32)
            nc.scalar.activation(scrap[:], zz[:], AF.Sin)

            # blob loads in consumption order: one descriptor unblocks the
            # whole uh-ch0 chain (consts + hT + Uc-ch0)
            blob0 = consts.tile([128, B0W], bf16)
            nc.sync.dma_start(out=blob0[:], in_=b0_e[:])
            uc1 = consts.tile([128, HCN, 128], bf16)
            nc.sync.dma_start(out=uc1[:], in_=uc1_e[:])
            blob1 = consts.tile([128, B1W], bf16)
            nc.gpsimd.dma_start(out=blob1[:], in_=b1_e[:])
            wc1t = consts.tile([128, FCN, 128], bf16)
            nc.gpsimd.dma_start(out=wc1t[:], in_=wc1_e[:])
            vN = consts.tile([128, BL, F], bf16)
            nc.gpsimd.dma_start(out=vN[:], in_=vN_e[:])

            bc = blob0[:, 0:4].bitcast(f32)
            om2 = blob0[:, 4 : 4 + 2 * K].rearrange("p (k t) -> p k t", t=2)
            wc2 = blob0[:, 4 + 2 * K : CW].rearrange(
                "p (c k t) -> p c k t", c=DCN, t=2)
            hT = blob0[:, CW : CW + HCN * BL * TH].rearrange(
                "p (hc b s) -> p hc b s", hc=HCN, b=BL)
            uc0 = blob0[:, CW + HCN * BL * TH :].rearrange(
                "p (hc d) -> p hc d", hc=HCN)
            vT = blob1[:, : FCN * BL * TV].rearrange(
                "p (fc b t) -> p fc b t", fc=FCN, b=BL)
            wc0 = blob1[:, FCN * BL * TV :].rearrange(
                "p (fc d) -> p fc d", fc=FCN)

            uh_sb = work.tile([128, DCN, BL * TH], bf16)
            wv_sb = work.tile([128, DCN, BL * TV], bf16)
            XA = work.tile([128, DCN, K, BL * TH], bf16)
            XB = work.tile([128, DCN, K, BL * TV], bf16)
            tsA = work.tile([128, DCN, K, BL * TH], bf16)
            tsB = work.tile([128, DCN, K, BL * TV], bf16)
            rA = work.tile([128, DCN, 2, K, BL * TH], bf16)
            rB = work.tile([128, DCN, 2, K, BL * TV], bf16)
            tgA = work.tile([128, DCN, 2, K, BL * TH], bf16)
            tgB = work.tile([128, DCN, 2, K, BL * TV], bf16)
            twA = work.tile([128, DCN, 2, K, BL * TH], bf16)
            Tt = work.tile([128, BL, TH], f32)
            gm = work.tile([128, BL, TH], f32)
            gr = work.tile([128, BL, TH], f32)
            gp = work.tile([128, BL, TH], f32)
            eT = work.tile([128, BL, TH], bf16)
            rden = work.tile([TH, BL], f32)
            usb = work.tile([TH, BL, F], f32)

            # ------------- per-chunk: proj -> X -> wrap -------------
            # Emission order = engine queue order; A-side chain completes
            # before any B-side DVE work so the first Sin fires early.
            def xbuild(dst, src_, width):
                nc.vector.tensor_tensor(
                    dst.rearrange("p k (q t) -> p k q t", t=2),
                    src_.rearrange("p (q t) -> p q t", t=2)
                    .unsqueeze(1)
                    .broadcast_to([128, K, width // 2, 2]),
                    om2.unsqueeze(2).broadcast_to([128, K, width // 2, 2]),
                    mybir.AluOpType.mult,
                )

            def wrap(ts, r, X):
                nc.vector.tensor_scalar(ts, X, MAGIC, None, AL.add)
                nc.vector.scalar_tensor_tensor(
                    r[:, 0], ts, -MAGIC, X, AL.add, AL.subtract
                )
                nc.vector.tensor_scalar(
                    r[:, 1].bitcast(mybir.dt.uint16),
                    r[:, 0].bitcast(mybir.dt.uint16),
                    0x7FFF, None, AL.bitwise_and,
                )

            def trig(tg, r):
                nc.scalar.activation(tg[:, 0], r[:, 0], AF.Sin, scale=-2 * PI)
                nc.scalar.activation(
                    tg[:, 1], r[:, 1], AF.Sin, bias=pihalf[:], scale=-2 * PI
                )

            def wcscale(ch, v_):
                nc.vector.tensor_tensor(
                    twA[:, ch, v_].rearrange("p k (q t) -> p k q t", t=2),
                    tgA[:, ch, v_].rearrange("p k (q t) -> p k q t", t=2),
                    wc2[:, ch].unsqueeze(2)
                    .broadcast_to([128, K, BL * TH // 2, 2]),
                    mybir.AluOpType.mult,
                )

            for ch in range(DCN):
                uh_ps = ps_pu.tile([128, BL * TH], f32, tag="uh")
                for hc in range(HCN):
                    nc.tensor.matmul(
                        uh_ps[:],
                        lhsT=(uc0[:, hc] if ch == 0 else uc1[:, hc]),
                        rhs=hT[:, hc],
                        start=(hc == 0),
                        stop=(hc == HCN - 1),
                    )
                # uh + b, cast bf16 (ACT Identity: func(x + bias))
                nc.scalar.activation(
                    uh_sb[:, ch, :], uh_ps[:], AF.Identity,
                    bias=bc[:, ch : ch + 1],
                )
                wv_ps = ps_pw.tile([128, BL * TV], f32, tag="wv")
                for fc in range(FCN):
                    nc.tensor.matmul(
                        wv_ps[:],
                        lhsT=(wc0[:, fc] if ch == 0 else wc1t[:, fc]),
                        rhs=vT[:, fc],
                        start=(fc == 0),
                        stop=(fc == FCN - 1),
                    )
                nc.scalar.copy(wv_sb[:, ch, :], wv_ps[:])
                xbuild(XA[:, ch], uh_sb[:, ch, :], BL * TH)
                xbuild(XB[:, ch], wv_sb[:, ch, :], BL * TV)
                wrap(tsA[:, ch], rA[:, ch], XA[:, ch])
                wrap(tsB[:, ch], rB[:, ch], XB[:, ch])
                trig(tgA[:, ch], rA[:, ch])
                trig(tgB[:, ch], rB[:, ch])

            # wc-scale the (smaller) A-side trig: w_d * C_k
            for ch in range(DCN):
                wcscale(ch, 0)
                wcscale(ch, 1)

            # ------------- q contraction over (d, k) -------------
            # qT[t, s] += sinA_w^T cosB + cosA_w^T sinB  per (ch, k)
            qps = []
            for b in range(BL):
                qp = ps_q.tile([128, TH], f32, tag="q", name=f"q{b}")
                qps.append(qp)
                n = 0
                for ch in range(DCN):
                    for k_ in range(K):
                        for va, vb in ((0, 1), (1, 0)):
                            nc.tensor.matmul(
                                qp[:],
                                lhsT=tgB[:, ch, vb, k_, b * TV : (b + 1) * TV],
                                rhs=twA[:, ch, va, k_, b * TH : (b + 1) * TH],
                                start=(n == 0),
                                stop=(n == 2 * DCN * K - 1),
                            )
                            n += 1

            # ------------- softmax + context per batch -------------
            for b in range(BL):
                # softmax weights via tanh (stays in the Sin table set):
                # e^q proportional-to (1+T)/(1-T), T = tanh(q/2)
                nc.scalar.activation(Tt[:, b, :], qps[b][:], AF.Tanh, scale=0.5)
                nc.vector.tensor_scalar(
                    gm[:, b, :], Tt[:, b, :], -1.0, 1.0, AL.mult, AL.add
                )
                nc.vector.tensor_scalar(
                    gp[:, b, :], Tt[:, b, :], 1.0, 1.0, AL.mult, AL.add
                )
                nc.vector.reciprocal(gr[:, b, :], gm[:, b, :])
                nc.vector.tensor_tensor(
                    eT[:, b, :], gp[:, b, :], gr[:, b, :], AL.mult
                )
                den_ps = ps_d.tile([TH, 1], f32, tag="den")
                nc.tensor.matmul(
                    den_ps[:], lhsT=eT[:, b, :], rhs=ones_t[:],
                    start=True, stop=True,
                )
                nc.vector.reciprocal(rden[:, b : b + 1], den_ps[:])
                ups = ps_u.tile([TH, F], f32, tag="u")
                nc.tensor.matmul(
                    ups[:], lhsT=eT[:, b, :], rhs=vN[:, b, :],
                    start=True, stop=True,
                )
                nc.scalar.mul(usb[:, b, :], ups[:], rden[:, b : b + 1])
                nc.sync.dma_start(out=out_e[b], in_=usb[:, b, :])

    _split_excess_waits(nc, mybir)
    return nc


def _get_nc():
    if "nc" not in _CACHE:
        _CACHE["nc"] = _build_nc()
    return _CACHE["nc"]


def _in_maps(v, h, W, U, b, w):
    """Host-side staging: shard over B and pre-arrange every operand into its
    on-chip layout and compute dtype."""
    v = np.asarray(v, dtype=np.float32)
    h = np.asarray(h, dtype=np.float32)
    W = np.asarray(W, dtype=np.float32)
    U = np.asarray(U, dtype=np.float32)
    b = np.asarray(b, dtype=np.float32)
    w = np.asarray(w, dtype=np.float32)

    Uc = np.ascontiguousarray(
        U.reshape(HCN, 128, DCN, 128).transpose(1, 0, 2, 3).astype(BF16)
    )
    Wc = np.ascontiguousarray(
        W.reshape(FCN, 128, DCN, 128).transpose(1, 0, 2, 3).astype(BF16)
    )
    bc = np.ascontiguousarray(b.reshape(DCN, 128).T)  # [128, DCN] f32
    om2 = np.ascontiguousarray(
        np.broadcast_to(OMP[None, :, None], (128, K, 2)).astype(BF16)
    )
    wdc = (w[:, 0][:, None] * C[None, :]).reshape(DCN, 128, K).transpose(1, 0, 2)
    wc2 = np.ascontiguousarray(
        np.broadcast_to(wdc[:, :, :, None], (128, DCN, K, 2)).astype(BF16)
    )
    cstb = np.concatenate([
        bc.astype(np.float32).view(BF16).reshape(128, 4),
        om2.reshape(128, 2 * K),
        wc2.reshape(128, 2 * DCN * K),
    ], axis=1)  # [128, CW] bf16

    maps = []
    for i in range(NCORES):
        vs = v[i * BL : (i + 1) * BL]  # [BL, TV, F]
        hs = h[i * BL : (i + 1) * BL]  # [BL, TH, H]
        hT = hs.transpose(2, 0, 1).reshape(HCN, 128, BL, TH) \
            .transpose(1, 0, 2, 3).astype(BF16)
        vTl = vs.transpose(2, 0, 1).reshape(FCN, 128, BL, TV) \
            .transpose(1, 0, 2, 3).astype(BF16)
        vNl = np.ascontiguousarray(vs.transpose(1, 0, 2).astype(BF16))
        b0 = np.concatenate([
            cstb, hT.reshape(128, -1), Uc[:, :, 0].reshape(128, -1)
        ], axis=1)
        b1 = np.concatenate([
            vTl.reshape(128, -1), Wc[:, :, 0].reshape(128, -1)
        ], axis=1)
        maps.append({
            "b0": np.ascontiguousarray(b0),
            "uc1": np.ascontiguousarray(Uc[:, :, 1]),
            "b1": np.ascontiguousarray(b1),
            "wc1": np.ascontiguousarray(Wc[:, :, 1]),
            "vN": vNl,
        })
    return maps


def _run(in_maps, trace=False, tmpdir=None):
    from concourse.bass_utils import run_bass_kernel_spmd

    nc = _get_nc()
    return run_bass_kernel_spmd(
        nc, in_maps, core_ids=list(range(NCORES)), trace=trace, tmpdir=tmpdir
    )


def kernel(v, h, W, U, b, w):
    res = _run(_in_maps(v, h, W, U, b, w), trace=False)
    return np.concatenate([res.results[i]["out"] for i in range(NCORES)], axis=0)


def _install_ntff_hook():
    import sys
    import types

    try:
        from antenv.axon_hooks import get_axon_ntff_profile_hook  # noqa: F401
        return
    except ImportError:
        pass
    import antenv
    from trn_agent_boot.trn_boot import _ntff_profile_via_ctypes

    mod = types.ModuleType("antenv.axon_hooks")
    state = {"hook": _ntff_profile_via_ctypes("/opt/axon/libaxon_pjrt.so")}
    mod.set_axon_ntff_profile_hook = lambda h: state.__setitem__("hook", h)
    mod.get_axon_ntff_profile_hook = lambda: state["hook"]
    sys.modules["antenv.axon_hooks"] = mod
    antenv.axon_hooks = mod


def kernel_traced(v, h, W, U, b, w, tmpdir=None):
    """Returns (output, exec_time_ns) using the NTFF profile path."""
    _install_ntff_hook()
    import concourse.bass_utils as bu

    bu.upload_artifacts = lambda d: str(d)
    res = _run(_in_maps(v, h, W, U, b, w), trace=True, tmpdir=tmpdir)
    out = np.concatenate([res.results[i]["out"] for i in range(NCORES)], axis=0)
    return out, res.exec_time_ns


# revision 3
# speedup vs baseline: 1.0290x; 1.0290x over previous
"""Additive (Bahdanau) attention kernel for Trainium2, 8 NeuronCores — v2.

Math (per batch b):
  Wv = v @ W            [Tv, D]
  Uh = h @ U            [Th, D]
  q[s,t] = sum_d w[d] * tanh(Uh[s,d] + Wv[t,d] + b[d])
  beta = softmax_t(q)
  u = beta @ v          [Th, F]

v2 core idea: replace tanh with a K-term sine fit
  tanh(x) ~= sum_k C_k sin(OM_k x)
and expand sin(OM(A+B)) = sin(OM A)cos(OM B) + cos(OM A)sin(OM B).  The huge
[s,t,d] broadcast+tanh tensor (the v1 bottleneck: ~27us ScalarE + ~17us DVE
per core) collapses into per-side trig evals on [s,d]x K and [t,d]x K plus
TensorE contractions over (d,k).  q lands in PSUM as [t,s], so softmax+context
need no transposes.

HW Sin is only valid on ~[-pi,pi], so arguments (up to ~21 rad) are
range-reduced in period units with the bf16 magic-number rounding trick:
  tmp  = bf16(Xp + (192+phi))      # write-cast rounds to integer quantum 1
  -r   = (tmp - (192+phi)) - Xp    # scalar_tensor_tensor
  trig = Sin(-r, scale=-2pi)       # = sin(2pi(Xp+phi) - 2pi round(.))
phi = 0 gives sin, phi = 0.25 gives cos.  All on DVE at 4x/2x bf16 rates.

Sharding: pure data-parallel over B (16 batches -> 2 per core), weights
replicated, no collectives.  Host-side staging ships every operand
pre-layouted and pre-cast (bf16 except f32 biases).
"""

import ml_dtypes
import numpy as np

B, TV, TH, F, H, D = 16, 128, 64, 512, 512, 256
NCORES = 8
BL = B // NCORES  # 2 batches per core
DCN = 2  # d chunks of 128
FCN = 4
HCN = 4
K = 5

# tanh(x) ~= sum C_k sin(2*pi*OMP_k x); free-frequency fit tuned end-to-end
# under exact kernel numerics.  OMP (period units) is bf16-exact.
OMP = np.array([0.04833984375, 0.099609375, 0.1826171875, 0.302734375,
                0.451171875], np.float32)
C = np.array([1.116507887840271, 0.22106504440307617, 0.2375125288963318,
              0.08239565789699554, 0.025441491976380348], np.float32)
PI = float(np.pi)
MAGIC = 192.0

_CACHE = {}
BF16 = ml_dtypes.bfloat16


def _split_excess_waits(nc, mybir):
    """Walrus rejects instructions carrying >1 sync-wait; split extras onto
    same-engine NoOp carriers."""
    EXEMPT = ("InstUnconditionalBranch", "InstCall")
    k = 0
    for f in nc.m.functions:
        for blk in f.blocks:
            insts = list(blk.instructions)
            out, changed = [], False
            for inst in insts:
                si = inst.sync_info
                tn = type(inst).__name__
                if (si is not None and si.on_wait and len(si.on_wait) > 1
                        and tn not in EXEMPT):
                    waits = list(si.on_wait)
                    for wext in waits[:-1]:
                        noop = mybir.InstNoOp(name=f"wsplit-{k}")
                        k += 1
                        noop.engine = inst.engine
                        noop.sync_info = mybir.SyncInfo(
                            on_wait=[wext], on_update=[]
                        )
                        out.append(noop)
                    inst.sync_info = mybir.SyncInfo(
                        on_wait=waits[-1:], on_update=list(si.on_update or [])
                    )
                    changed = True
                out.append(inst)
            if changed:
                blk.instructions = out


def _build_nc():
    import concourse.bass as bass
    import concourse.tile as tile
    from concourse import mybir

    f32 = mybir.dt.float32
    bf16 = mybir.dt.bfloat16
    AF = mybir.ActivationFunctionType
    AL = mybir.AluOpType

    CW = 2 * (2 + K + DCN * K)  # consts width in bf16 units
    B0W = CW + HCN * BL * TH + HCN * 128   # consts | hT | Uc-ch0
    B1W = FCN * BL * TV + FCN * 128        # vT | Wc-ch0
    nc = bass.Bass()
    b0_e = nc.declare_dram_parameter("b0", [128, B0W], bf16, isOutput=False)
    uc1_e = nc.declare_dram_parameter("uc1", [128, HCN, 128], bf16, isOutput=False)
    b1_e = nc.declare_dram_parameter("b1", [128, B1W], bf16, isOutput=False)
    wc1_e = nc.declare_dram_parameter("wc1", [128, FCN, 128], bf16, isOutput=False)
    vN_e = nc.declare_dram_parameter("vN", [128, BL, F], bf16, isOutput=False)
    out_e = nc.declare_dram_parameter("out", [BL, TH, F], f32, isOutput=True)

    with tile.TileContext(nc) as tc:
        with (
            tc.tile_pool(name="consts", bufs=1) as consts,
            tc.tile_pool(name="work", bufs=1) as work,
            tc.tile_pool(name="ps_pu", bufs=1, space="PSUM") as ps_pu,
            tc.tile_pool(name="ps_pw", bufs=1, space="PSUM") as ps_pw,
            tc.tile_pool(name="ps_q", bufs=2, space="PSUM") as ps_q,
            tc.tile_pool(name="ps_u", bufs=2, space="PSUM") as ps_u,
            tc.tile_pool(name="ps_d", bufs=1, space="PSUM") as ps_d,
        ):
            # ---------------- loads ----------------
            zz = consts.tile([128, 1], f32)
            nc.gpsimd.memset(zz[:], 0.0)
            ones_t = consts.tile([128, 1], bf16)
            nc.gpsimd.memset(ones_t[:], 1.0)
            pihalf = consts.tile([128, 1], f32)
            nc.gpsimd.memset(pihalf[:], PI / 2)
            # touch ACT so the Sin table set loads off the critical path
            scrap = consts.tile([128, 1], f32)
            nc.scalar.activation(scrap[:], zz[:], AF.Sin)

            # blob loads in consumption order: one descriptor unblocks the
            # whole uh-ch0 chain (consts + hT + Uc-ch0)
            blob0 = consts.tile([128, B0W], bf16)
            nc.sync.dma_start(out=blob0[:], in_=b0_e[:])
            uc1 = consts.tile([128, HCN, 128], bf16)
            nc.sync.dma_start(out=uc1[:], in_=uc1_e[:])
            blob1 = consts.tile([128, B1W], bf16)
            nc.gpsimd.dma_start(out=blob1[:], in_=b1_e[:])
            wc1t = consts.tile([128, FCN, 128], bf16)
            nc.gpsimd.dma_start(out=wc1t[:], in_=wc1_e[:])
            vN = consts.tile([128, BL, F], bf16)
            nc.gpsimd.dma_start(out=vN[:], in_=vN_e[:])

            bc = blob0[:, 0:4].bitcast(f32)
            om2 = blob0[:, 4 : 4 + 2 * K].rearrange("p (k t) -> p k t", t=2)
            wc2 = blob0[:, 4 + 2 * K : CW].rearrange(
                "p (c k t) -> p c k t", c=DCN, t=2)
            hT = blob0[:, CW : CW + HCN * BL * TH].rearrange(
                "p (hc b s) -> p hc b s", hc=HCN, b=BL)
            uc0 = blob0[:, CW + HCN * BL * TH :].rearrange(
                "p (hc d) -> p hc d", hc=HCN)
            vT = blob1[:, : FCN * BL * TV].rearrange(
                "p (fc b t) -> p fc b t", fc=FCN, b=BL)
            wc0 = blob1[:, FCN * BL * TV :].rearrange(
                "p (fc d) -> p fc d", fc=FCN)

            uh_sb = work.tile([128, DCN, BL * TH], bf16)
            wv_sb = work.tile([128, DCN, BL * TV], bf16)
            XA = work.tile([128, DCN, K, BL * TH], bf16)
            XB = work.tile([128, DCN, K, BL * TV], bf16)
            tsA = work.tile([128, DCN, K, BL * TH], bf16)
            tsB = work.tile([128, DCN, K, BL * TV], bf16)
            rA = work.tile([128, DCN, 2, K, BL * TH], bf16)
            rB = work.tile([128, DCN, 2, K, BL * TV], bf16)
            tgA = work.tile([128, DCN, 2, K, BL * TH], bf16)
            tgB = work.tile([128, DCN, 2, K, BL * TV], bf16)
            twA = work.tile([128, DCN, 2, K, BL * TH], bf16)
            Tt = work.tile([128, BL, TH], f32)
            gm = work.tile([128, BL, TH], f32)
            gr = work.tile([128, BL, TH], f32)
            gp = work.tile([128, BL, TH], f32)
            eT = work.tile([128, BL, TH], bf16)
            rden = work.tile([TH, BL], f32)
            usb = work.tile([TH, BL, F], f32)

            # ------------- per-chunk: proj -> X -> wrap -------------
            # Emission order = engine queue order; A-side chain completes
            # before any B-side DVE work so the first Sin fires early.
            def xbuild(dst, src_, width):
                nc.vector.tensor_tensor(
                    dst.rearrange("p k (q t) -> p k q t", t=2),
                    src_.rearrange("p (q t) -> p q t", t=2)
                    .unsqueeze(1)
                    .broadcast_to([128, K, width // 2, 2]),
                    om2.unsqueeze(2).broadcast_to([128, K, width // 2, 2]),
                    mybir.AluOpType.mult,
                )

            def wrap(ts, r, X):
                nc.vector.tensor_scalar(ts, X, MAGIC, None, AL.add)
                nc.vector.scalar_tensor_tensor(
                    r[:, 0], ts, -MAGIC, X, AL.add, AL.subtract
                )
                nc.vector.tensor_scalar(
                    r[:, 1].bitcast(mybir.dt.uint16),
                    r[:, 0].bitcast(mybir.dt.uint16),
                    0x7FFF, None, AL.bitwise_and,
                )

            def trig(tg, r):
                nc.scalar.activation(tg[:, 0], r[:, 0], AF.Sin, scale=-2 * PI)
                nc.scalar.activation(
                    tg[:, 1], r[:, 1], AF.Sin, bias=pihalf[:], scale=-2 * PI
                )

            def wcscale(ch, v_):
                nc.vector.tensor_tensor(
                    twA[:, ch, v_].rearrange("p k (q t) -> p k q t", t=2),
                    tgA[:, ch, v_].rearrange("p k (q t) -> p k q t", t=2),
                    wc2[:, ch].unsqueeze(2)
                    .broadcast_to([128, K, BL * TH // 2, 2]),
                    mybir.AluOpType.mult,
                )

            def uhproj(ch):
                uh_ps = ps_pu.tile([128, BL * TH], f32, tag="uh")
                for hc in range(HCN):
                    nc.tensor.matmul(
                        uh_ps[:],
                        lhsT=(uc0[:, hc] if ch == 0 else uc1[:, hc]),
                        rhs=hT[:, hc],
                        start=(hc == 0),
                        stop=(hc == HCN - 1),
                    )
                # uh + b, cast bf16 (ACT Identity: func(x + bias))
                nc.scalar.activation(
                    uh_sb[:, ch, :], uh_ps[:], AF.Identity,
                    bias=bc[:, ch : ch + 1],
                )

            def wvproj(ch):
                wv_ps = ps_pw.tile([128, BL * TV], f32, tag="wv")
                for fc in range(FCN):
                    nc.tensor.matmul(
                        wv_ps[:],
                        lhsT=(wc0[:, fc] if ch == 0 else wc1t[:, fc]),
                        rhs=vT[:, fc],
                        start=(fc == 0),
                        stop=(fc == FCN - 1),
                    )
                return wv_ps

            def aside(ch):
                xbuild(XA[:, ch], uh_sb[:, ch, :], BL * TH)
                wrap(tsA[:, ch], rA[:, ch], XA[:, ch])
                trig(tgA[:, ch], rA[:, ch])

            def bside(ch):
                xbuild(XB[:, ch], wv_sb[:, ch, :], BL * TV)
                wrap(tsB[:, ch], rB[:, ch], XB[:, ch])
                trig(tgB[:, ch], rB[:, ch])

            # Emission = queue order.  ACT runs Id0, sinA0, cosA0, copy0,
            # Id1, copy1, then the remaining six Sins back-to-back; the
            # wv copies never head-block a ready Sin.
            uhproj(0)
            wv0 = wvproj(0)
            aside(0)
            wcscale(0, 0)
            wcscale(0, 1)
            nc.scalar.copy(wv_sb[:, 0, :], wv0[:])
            uhproj(1)
            wv1 = wvproj(1)
            nc.scalar.copy(wv_sb[:, 1, :], wv1[:])
            bside(0)
            aside(1)
            bside(1)
            wcscale(1, 0)
            wcscale(1, 1)

            # ------------- q contraction over (d, k) -------------
            # qT[t, s] += sinA_w^T cosB + cosA_w^T sinB  per (ch, k)
            qps = []
            for b in range(BL):
                qp = ps_q.tile([128, TH], f32, tag="q", name=f"q{b}")
                qps.append(qp)
                n = 0
                for ch in range(DCN):
                    for k_ in range(K):
                        for va, vb in ((0, 1), (1, 0)):
                            nc.tensor.matmul(
                                qp[:],
                                lhsT=tgB[:, ch, vb, k_, b * TV : (b + 1) * TV],
                                rhs=twA[:, ch, va, k_, b * TH : (b + 1) * TH],
                                start=(n == 0),
                                stop=(n == 2 * DCN * K - 1),
                            )
                            n += 1

            # ------------- softmax + context per batch -------------
            for b in range(BL):
                # softmax weights via tanh (stays in the Sin table set):
                # e^q proportional-to (1+T)/(1-T), T = tanh(q/2)
                nc.scalar.activation(Tt[:, b, :], qps[b][:], AF.Tanh, scale=0.5)
                nc.vector.tensor_scalar(
                    gm[:, b, :], Tt[:, b, :], -1.0, 1.0, AL.mult, AL.add
                )
                nc.vector.tensor_scalar(
                    gp[:, b, :], Tt[:, b, :], 1.0, 1.0, AL.mult, AL.add
                )
                nc.vector.reciprocal(gr[:, b, :], gm[:, b, :])
                nc.vector.tensor_tensor(
                    eT[:, b, :], gp[:, b, :], gr[:, b, :], AL.mult
                )
                den_ps = ps_d.tile([TH, 1], f32, tag="den")
                nc.tensor.matmul(
                    den_ps[:], lhsT=eT[:, b, :], rhs=ones_t[:],
                    start=True, stop=True,
                )
                nc.vector.reciprocal(rden[:, b : b + 1], den_ps[:])
                ups = ps_u.tile([TH, F], f32, tag="u")
                nc.tensor.matmul(
                    ups[:], lhsT=eT[:, b, :], rhs=vN[:, b, :],
                    start=True, stop=True,
                )
                nc.scalar.mul(usb[:, b, :], ups[:], rden[:, b : b + 1])
                nc.sync.dma_start(out=out_e[b], in_=usb[:, b, :])

    _split_excess_waits(nc, mybir)
    return nc


def _get_nc():
    if "nc" not in _CACHE:
        _CACHE["nc"] = _build_nc()
    return _CACHE["nc"]


def _in_maps(v, h, W, U, b, w):
    """Host-side staging: shard over B and pre-arrange every operand into its
    on-chip layout and compute dtype."""
    v = np.asarray(v, dtype=np.float32)
    h = np.asarray(h, dtype=np.float32)
    W = np.asarray(W, dtype=np.float32)
    U = np.asarray(U, dtype=np.float32)
    b = np.asarray(b, dtype=np.float32)
    w = np.asarray(w, dtype=np.float32)

    Uc = np.ascontiguousarray(
        U.reshape(HCN, 128, DCN, 128).transpose(1, 0, 2, 3).astype(BF16)
    )
    Wc = np.ascontiguousarray(
        W.reshape(FCN, 128, DCN, 128).transpose(1, 0, 2, 3).astype(BF16)
    )
    bc = np.ascontiguousarray(b.reshape(DCN, 128).T)  # [128, DCN] f32
    om2 = np.ascontiguousarray(
        np.broadcast_to(OMP[None, :, None], (128, K, 2)).astype(BF16)
    )
    wdc = (w[:, 0][:, None] * C[None, :]).reshape(DCN, 128, K).transpose(1, 0, 2)
    wc2 = np.ascontiguousarray(
        np.broadcast_to(wdc[:, :, :, None], (128, DCN, K, 2)).astype(BF16)
    )
    cstb = np.concatenate([
        bc.astype(np.float32).view(BF16).reshape(128, 4),
        om2.reshape(128, 2 * K),
        wc2.reshape(128, 2 * DCN * K),
    ], axis=1)  # [128, CW] bf16

    maps = []
    for i in range(NCORES):
        vs = v[i * BL : (i + 1) * BL]  # [BL, TV, F]
        hs = h[i * BL : (i + 1) * BL]  # [BL, TH, H]
        hT = hs.transpose(2, 0, 1).reshape(HCN, 128, BL, TH) \
            .transpose(1, 0, 2, 3).astype(BF16)
        vTl = vs.transpose(2, 0, 1).reshape(FCN, 128, BL, TV) \
            .transpose(1, 0, 2, 3).astype(BF16)
        vNl = np.ascontiguousarray(vs.transpose(1, 0, 2).astype(BF16))
        b0 = np.concatenate([
            cstb, hT.reshape(128, -1), Uc[:, :, 0].reshape(128, -1)
        ], axis=1)
        b1 = np.concatenate([
            vTl.reshape(128, -1), Wc[:, :, 0].reshape(128, -1)
        ], axis=1)
        maps.append({
            "b0": np.ascontiguousarray(b0),
            "uc1": np.ascontiguousarray(Uc[:, :, 1]),
            "b1": np.ascontiguousarray(b1),
            "wc1": np.ascontiguousarray(Wc[:, :, 1]),
            "vN": vNl,
        })
    return maps


def _run(in_maps, trace=False, tmpdir=None):
    from concourse.bass_utils import run_bass_kernel_spmd

    nc = _get_nc()
    return run_bass_kernel_spmd(
        nc, in_maps, core_ids=list(range(NCORES)), trace=trace, tmpdir=tmpdir
    )


def kernel(v, h, W, U, b, w):
    res = _run(_in_maps(v, h, W, U, b, w), trace=False)
    return np.concatenate([res.results[i]["out"] for i in range(NCORES)], axis=0)


def _install_ntff_hook():
    import sys
    import types

    try:
        from antenv.axon_hooks import get_axon_ntff_profile_hook  # noqa: F401
        return
    except ImportError:
        pass
    import antenv
    from trn_agent_boot.trn_boot import _ntff_profile_via_ctypes

    mod = types.ModuleType("antenv.axon_hooks")
    state = {"hook": _ntff_profile_via_ctypes("/opt/axon/libaxon_pjrt.so")}
    mod.set_axon_ntff_profile_hook = lambda h: state.__setitem__("hook", h)
    mod.get_axon_ntff_profile_hook = lambda: state["hook"]
    sys.modules["antenv.axon_hooks"] = mod
    antenv.axon_hooks = mod


def kernel_traced(v, h, W, U, b, w, tmpdir=None):
    """Returns (output, exec_time_ns) using the NTFF profile path."""
    _install_ntff_hook()
    import concourse.bass_utils as bu

    bu.upload_artifacts = lambda d: str(d)
    res = _run(_in_maps(v, h, W, U, b, w), trace=True, tmpdir=tmpdir)
    out = np.concatenate([res.results[i]["out"] for i in range(NCORES)], axis=0)
    return out, res.exec_time_ns
